# revision 1
# baseline (speedup 1.0000x reference)
"""Trainium2 Bass kernel for nn_LinkPredictor (2-layer GCN + edge-dot decode).

Strategy (8 NeuronCores, SPMD):
  - Nodes sharded: core c owns rows [c*12544, (c+1)*12544) of the padded
    node table (N=100000 padded to 100352 = 8*98*128).
  - Edges assigned to the core owning their dst. Per core, edges are grouped
    by (dst window of 128 nodes, src chunk of 25088 rows) with a uniform slot
    budget B per group (padded with dead slots, norm=0) so all 8 cores run an
    identical program.
  - GCN norm (dinv[s]*dinv[d]) is folded into one-hot selection matrices S
    built on-device by a dual-op tensor_scalar: S[e,:] = (iota==dstloc[e])*norm[e].
  - Message gather: dma_gather (GPSIMD SWDGE, 4 queues) from a bf16 node table
    in DRAM; segment-sum = PE matmul accumulation S^T @ M into PSUM (fp32).
  - Self-loops handled separately (dinv^2 * h[n], no gather).
  - Inter-layer full-table exchange via AllGather collectives.
  - Decode: gather z2[s], z2[d], DVE multiply + reduce.
"""
import contextlib
import math
import numpy as np
import ml_dtypes

import concourse.bass as bass
import concourse.tile as tile
from concourse import bacc, mybir
from concourse.bass_utils import run_bass_kernel_spmd
from concourse.tile_rust import add_dep_helper

F32 = mybir.dt.float32
BF16 = mybir.dt.bfloat16
I16 = mybir.dt.int16
BF = ml_dtypes.bfloat16


class Cfg:
    def __init__(self, N=100000, E=1600000, EL=100000, D=128, ncores=8,
                 nw=98, nchunks=4, wb=4):
        self.N, self.E, self.EL, self.D, self.NC = N, E, EL, D, ncores
        self.NW = nw                      # windows (128 nodes each) per core
        self.SHARD = nw * 128             # nodes per core (padded)
        self.NP = self.SHARD * ncores     # padded node count
        assert self.NP >= N
        self.NCH = nchunks                # src chunks (int16 index range)
        assert self.NP % nchunks == 0
        self.CHROWS = self.NP // nchunks
        assert self.CHROWS <= 32768
        self.WB = wb                      # windows per gather/aggregate batch
        self.NBATCH = math.ceil(nw / wb)


DEFAULT = Cfg()


def _wrap_idxs(idx):
    """[n] ints -> [128, n//16] int16 wrapped in 16 partitions, replicated 8x."""
    n = len(idx)
    assert n % 16 == 0
    w = np.asarray(idx, dtype=np.int16).reshape(n // 16, 16).T
    return np.ascontiguousarray(np.tile(w, (8, 1)))


def host_prep(cfg, x, edge_index, edge_label_index, W1, b1, W2, b2):
    """All host-side sharding/layout. Returns (per-core input maps, meta)."""
    c = cfg
    src = np.asarray(edge_index[0], dtype=np.int64)
    dst = np.asarray(edge_index[1], dtype=np.int64)
    deg = np.bincount(dst, minlength=c.N).astype(np.float64) + 1.0
    dinv = 1.0 / np.sqrt(deg)                      # [N]
    dinv_p = np.ones(c.NP, dtype=np.float64)
    dinv_p[:c.N] = dinv
    norm_e = (dinv[src] * dinv[dst]).astype(np.float32)

    core_of = dst // c.SHARD
    w_of = (dst - core_of * c.SHARD) // 128
    ch_of = src // c.CHROWS

    key = (core_of * c.NW + w_of) * c.NCH + ch_of
    order = np.argsort(key, kind="stable")
    ngroups = c.NC * c.NW * c.NCH
    counts = np.bincount(key[order], minlength=ngroups)
    B = int(128 * math.ceil(max(int(counts.max()), 1) / 128))
    starts = np.zeros(ngroups + 1, dtype=np.int64)
    np.cumsum(counts, out=starts[1:])

    TPG = B // 128                       # tiles per group
    TOT = c.NW * c.NCH * B               # slots per core per layer
    TOT_TILES = TOT // 128

    # global slot order per core: batch b -> chunk ch -> window w (in batch) -> i
    idx_arr = np.zeros((c.NC, TOT), dtype=np.int64)
    dstloc_arr = np.zeros((c.NC, TOT), dtype=np.float32)
    norm_arr = np.zeros((c.NC, TOT), dtype=np.float32)
    for core in range(c.NC):
        pos = 0
        for b in range(c.NBATCH):
            wlo, whi = b * c.WB, min((b + 1) * c.WB, c.NW)
            for ch in range(c.NCH):
                for w in range(wlo, whi):
                    g = (core * c.NW + w) * c.NCH + ch
                    eids = order[starts[g]:starts[g + 1]]
                    n = len(eids)
                    idx_arr[core, pos:pos + n] = src[eids] - ch * c.CHROWS
                    dstloc_arr[core, pos:pos + n] = (
                        dst[eids] - core * c.SHARD - w * 128)
                    norm_arr[core, pos:pos + n] = norm_e[eids]
                    pos += B
        assert pos == TOT

    # decode: label edge j -> core j // ELC; slots grouped by (chunk(s), chunk(d))
    assert c.EL % c.NC == 0
    ELC = c.EL // c.NC
    ls = np.asarray(edge_label_index[0], dtype=np.int64)
    ld = np.asarray(edge_label_index[1], dtype=np.int64)
    kd = (ls // c.CHROWS) * c.NCH + (ld // c.CHROWS)
    NG_DEC = c.NCH * c.NCH
    B_dec = 0
    for core in range(c.NC):
        cnt = np.bincount(kd[core * ELC:(core + 1) * ELC], minlength=NG_DEC)
        B_dec = max(B_dec, int(cnt.max()))
    B_dec = 128 * math.ceil(max(B_dec, 1) / 128)
    TOT_DEC = NG_DEC * B_dec
    idx_s = np.zeros((c.NC, TOT_DEC), dtype=np.int64)
    idx_d = np.zeros((c.NC, TOT_DEC), dtype=np.int64)
    slot2j = np.full((c.NC, TOT_DEC), -1, dtype=np.int64)
    for core in range(c.NC):
        jlo = core * ELC
        kk = kd[jlo:jlo + ELC]
        o = np.argsort(kk, kind="stable")
        cnt = np.bincount(kk, minlength=NG_DEC)
        st = np.zeros(NG_DEC + 1, dtype=np.int64)
        np.cumsum(cnt, out=st[1:])
        for g in range(NG_DEC):
            js = o[st[g]:st[g + 1]] + jlo
            n = len(js)
            pos = g * B_dec
            idx_s[core, pos:pos + n] = ls[js] - (g // c.NCH) * c.CHROWS
            idx_d[core, pos:pos + n] = ld[js] - (g % c.NCH) * c.CHROWS
            slot2j[core, pos:pos + n] = js

    xp = np.zeros((c.NP, c.D), dtype=np.float32)
    xp[:c.N] = np.asarray(x, dtype=np.float32)
    dinv_f = dinv_p.astype(np.float32)
    in_maps = []
    for core in range(c.NC):
        sl = slice(core * c.SHARD, (core + 1) * c.SHARD)
        in_maps.append({
            "xT": np.ascontiguousarray(xp[sl].T).astype(BF),
            "W1": np.asarray(W1, dtype=np.float32).astype(BF),
            "W2": np.asarray(W2, dtype=np.float32).astype(BF),
            "b1r": np.tile(np.asarray(b1, np.float32)[None, :], (128, 1)),
            "b2r": np.tile(np.asarray(b2, np.float32)[None, :], (128, 1)),
            "dinv2": np.ascontiguousarray(
                (dinv_f[sl] ** 2).reshape(c.NW, 128).T),
            "gidx": _wrap_idxs(idx_arr[core]),
            "dstloc": np.ascontiguousarray(
                dstloc_arr[core].reshape(TOT_TILES, 128).T),
            "gnorm": np.ascontiguousarray(
                norm_arr[core].reshape(TOT_TILES, 128).T),
            "didx_s": _wrap_idxs(idx_s[core]),
            "didx_d": _wrap_idxs(idx_d[core]),
        })
    meta = dict(B=B, TPG=TPG, TOT=TOT, TOT_TILES=TOT_TILES,
                B_dec=B_dec, TOT_DEC=TOT_DEC, slot2j=slot2j)
    return in_maps, meta


def build_program(cfg, meta, num_cores=None):
    c = cfg
    NCores = num_cores or c.NC
    B, TPG, TOT, TOT_TILES = meta["B"], meta["TPG"], meta["TOT"], meta["TOT_TILES"]
    B_dec, TOT_DEC = meta["B_dec"], meta["TOT_DEC"]
    D = c.D

    nc = bacc.Bacc("TRN2", target_bir_lowering=False, debug=False,
                   num_devices=NCores, num_swdge_queues=min(4, c.NCH))
    NQ = min(4, c.NCH)

    xT_in = nc.dram_tensor("xT", [D, c.SHARD], BF16, kind="ExternalInput")
    W1_in = nc.dram_tensor("W1", [D, D], BF16, kind="ExternalInput")
    W2_in = nc.dram_tensor("W2", [D, D], BF16, kind="ExternalInput")
    b1_in = nc.dram_tensor("b1r", [128, D], F32, kind="ExternalInput")
    b2_in = nc.dram_tensor("b2r", [128, D], F32, kind="ExternalInput")
    dinv2_in = nc.dram_tensor("dinv2", [128, c.NW], F32, kind="ExternalInput")
    gidx_in = nc.dram_tensor("gidx", [128, TOT // 16], I16, kind="ExternalInput")
    dstloc_in = nc.dram_tensor("dstloc", [128, TOT_TILES], F32, kind="ExternalInput")
    gnorm_in = nc.dram_tensor("gnorm", [128, TOT_TILES], F32, kind="ExternalInput")
    didx_s_in = nc.dram_tensor("didx_s", [128, TOT_DEC // 16], I16, kind="ExternalInput")
    didx_d_in = nc.dram_tensor("didx_d", [128, TOT_DEC // 16], I16, kind="ExternalInput")
    dots_out = nc.dram_tensor("dots", [128, TOT_DEC // 128], F32, kind="ExternalOutput")

    gst = {"count": 0, "prev": None}

    def emit_gather(out_ap, in_ap, idx_ap, n_idx):
        q = gst["count"] % NQ
        inst = nc.gpsimd.dma_gather(out_ap, in_ap, idx_ap, n_idx, n_idx, D,
                                    queue_num=q, single_packet=False)
        if gst["prev"] is not None:
            add_dep_helper(inst.ins, gst["prev"].ins, sync=False,
                           reason="pin swdge queue order")
        gst["prev"] = inst
        gst["count"] += 1
        return inst

    shard1 = nc.dram_tensor("shard1", [c.SHARD, D], BF16)
    shard2 = nc.dram_tensor("shard2", [c.SHARD, D], BF16)
    shardz = nc.dram_tensor("shardz", [c.SHARD, D], BF16)
    table1 = nc.dram_tensor("table1", [c.NP, D], BF16)
    table2 = nc.dram_tensor("table2", [c.NP, D], BF16)
    tablez = nc.dram_tensor("tablez", [c.NP, D], BF16)

    iota_dram = nc.inline_tensor(
        np.tile(np.arange(128, dtype=np.float32), (128, 1)).astype(BF), "iota_c")
    ident_dram = nc.inline_tensor(np.eye(128, dtype=np.float32).astype(BF), "ident_c")

    cc_sem = nc.alloc_semaphore("cc_sem")
    core_ids = list(range(NCores))

    with tile.TileContext(nc) as tc:
        with contextlib.ExitStack() as es:
            const = es.enter_context(tc.tile_pool(name="const", bufs=1))
            meta_p = es.enter_context(tc.tile_pool(name="meta", bufs=1))

            w1_sb = const.tile([D, D], BF16); nc.sync.dma_start(w1_sb[:], W1_in[:])
            w2_sb = const.tile([D, D], BF16); nc.sync.dma_start(w2_sb[:], W2_in[:])
            b1_sb = const.tile([128, D], F32); nc.sync.dma_start(b1_sb[:], b1_in[:])
            b2_sb = const.tile([128, D], F32); nc.sync.dma_start(b2_sb[:], b2_in[:])
            dinv2_sb = const.tile([128, c.NW], F32)
            nc.sync.dma_start(dinv2_sb[:], dinv2_in[:])
            iota_sb = const.tile([128, 128], BF16)
            nc.sync.dma_start(iota_sb[:], iota_dram[:])
            ident_sb = const.tile([128, 128], BF16)
            nc.sync.dma_start(ident_sb[:], ident_dram[:])
            gidx_sb = meta_p.tile([128, TOT // 16], I16)
            nc.sync.dma_start(gidx_sb[:], gidx_in[:])
            dstloc_sb = meta_p.tile([128, TOT_TILES], F32)
            nc.sync.dma_start(dstloc_sb[:], dstloc_in[:])
            gnorm_sb = meta_p.tile([128, TOT_TILES], F32)
            nc.sync.dma_start(gnorm_sb[:], gnorm_in[:])

            def all_gather(shard, table_out, sem, n_before):
                tc.strict_bb_all_engine_barrier()
                with tc.tile_critical():
                    nc.gpsimd.collective_compute(
                        "AllGather", mybir.AluOpType.bypass,
                        replica_groups=[core_ids],
                        ins=[shard[:]], outs=[table_out[:]],
                    ).then_inc(cc_sem)
                    nc.gpsimd.wait_ge(cc_sem, n_before + 1)
                tc.strict_bb_all_engine_barrier()

            def layer(lid, table, h_tiles, bias_sb, shard_next, sem_next,
                      out_pool, make_next):
                out_tiles = []
                with tc.tile_pool(name=f"M{lid}", bufs=2) as Mp, \
                     tc.tile_pool(name=f"S{lid}", bufs=4) as Sp, \
                     tc.tile_pool(name=f"ag{lid}", bufs=4, space="PSUM") as agp, \
                     tc.tile_pool(name=f"tp{lid}", bufs=2, space="PSUM") as tpp, \
                     tc.tile_pool(name=f"ep{lid}", bufs=3) as epp:
                    for b in range(c.NBATCH):
                        wlo = b * c.WB
                        whi = min(wlo + c.WB, c.NW)
                        nwb = whi - wlo
                        cols_per_ch = nwb * TPG
                        Mt = Mp.tile([128, c.NCH * cols_per_ch, D], BF16, tag="M")
                        slot_base = wlo * c.NCH * B
                        for ch in range(c.NCH):
                            n_idx = nwb * B
                            off16 = (slot_base + ch * n_idx) // 16
                            emit_gather(
                                Mt[:, ch * cols_per_ch:(ch + 1) * cols_per_ch, :],
                                table[ch * c.CHROWS:(ch + 1) * c.CHROWS, :],
                                gidx_sb[:, off16:off16 + n_idx // 16],
                                n_idx)
                        tile_base = slot_base // 128
                        for wi in range(nwb):
                            w = wlo + wi
                            ps = agp.tile([128, D], F32, tag="agg")
                            nmm = c.NCH * TPG
                            k = 0
                            for ch in range(c.NCH):
                                for t in range(TPG):
                                    tcol = tile_base + (ch * nwb + wi) * TPG + t
                                    S = Sp.tile([128, 128], BF16, tag="S")
                                    nc.vector.tensor_scalar(
                                        S[:], iota_sb[:],
                                        dstloc_sb[:, tcol:tcol + 1],
                                        gnorm_sb[:, tcol:tcol + 1],
                                        mybir.AluOpType.is_equal,
                                        mybir.AluOpType.mult)
                                    mcol = (ch * nwb + wi) * TPG + t
                                    nc.tensor.matmul(
                                        ps[:], lhsT=S[:], rhs=Mt[:, mcol, :],
                                        start=(k == 0), stop=(k == nmm - 1))
                                    k += 1
                            selfz = epp.tile([128, D], F32, tag="selfz")
                            nc.vector.tensor_scalar(
                                selfz[:], h_tiles[w][:],
                                dinv2_sb[:, w:w + 1], None,
                                mybir.AluOpType.mult)
                            s1 = epp.tile([128, D], F32, tag="s1")
                            nc.vector.tensor_tensor(
                                s1[:], ps[:], selfz[:], op=mybir.AluOpType.add)
                            s2 = epp.tile([128, D], F32, tag="s2")
                            nc.vector.tensor_tensor(
                                s2[:], s1[:], bias_sb[:], op=mybir.AluOpType.add)
                            if make_next:
                                z = epp.tile([128, D], BF16, tag="z")
                                nc.scalar.activation(
                                    z[:], s2[:], mybir.ActivationFunctionType.Relu)
                                zt_ps = tpp.tile([128, D], BF16, tag="zt")
                                nc.tensor.transpose(zt_ps[:], z[:], ident_sb[:])
                                zT = epp.tile([128, D], BF16, tag="zT")
                                nc.vector.tensor_copy(zT[:], zt_ps[:])
                                h2ps = tpp.tile([128, D], F32, tag="h2")
                                nc.tensor.matmul(h2ps[:], lhsT=zT[:], rhs=w2_sb[:],
                                                 start=True, stop=True)
                                ht = out_pool.tile([128, D], BF16, tag="nxt")
                                nc.vector.tensor_copy(ht[:], h2ps[:])
                            else:
                                ht = out_pool.tile([128, D], BF16, tag="nxt")
                                nc.scalar.activation(
                                    ht[:], s2[:], mybir.ActivationFunctionType.Relu)
                            nc.sync.dma_start(
                                shard_next[w * 128:(w + 1) * 128, :], ht[:])
                            out_tiles.append(ht)
                return out_tiles

            with tc.tile_pool(name="hsb2", bufs=c.NW) as hsb2:
                with tc.tile_pool(name="hsb1", bufs=c.NW) as hsb1:
                    # P0: h1 = x @ W1 for own shard
                    h1_tiles = []
                    with tc.tile_pool(name="p0", bufs=3) as p0, \
                         tc.tile_pool(name="p0ps", bufs=2, space="PSUM") as p0ps:
                        for w in range(c.NW):
                            xt = p0.tile([D, 128], BF16)
                            nc.sync.dma_start(
                                xt[:], xT_in[:, w * 128:(w + 1) * 128])
                            ps = p0ps.tile([128, D], F32, tag="ps")
                            nc.tensor.matmul(ps[:], lhsT=xt[:], rhs=w1_sb[:],
                                             start=True, stop=True)
                            h1t = hsb1.tile([128, D], BF16, tag="h1t")
                            nc.vector.tensor_copy(h1t[:], ps[:])
                            nc.sync.dma_start(
                                shard1[w * 128:(w + 1) * 128, :], h1t[:])
                            h1_tiles.append(h1t)
                    all_gather(shard1, table1, None, 0)
                    h2_tiles = layer(1, table1, h1_tiles, b1_sb, shard2,
                                     None, hsb2, make_next=True)
                all_gather(shard2, table2, None, 1)
                with tc.tile_pool(name="zsink", bufs=3) as zsink:
                    layer(2, table2, h2_tiles, b2_sb, shardz,
                          None, zsink, make_next=False)
            all_gather(shardz, tablez, None, 2)

            # decode
            with tc.tile_pool(name="didx", bufs=1) as didxp, \
                 tc.tile_pool(name="dM", bufs=1) as dMp, \
                 tc.tile_pool(name="dw", bufs=4) as dwp, \
                 tc.tile_pool(name="dout", bufs=1) as doutp:
                ds_sb = didxp.tile([128, TOT_DEC // 16], I16)
                nc.sync.dma_start(ds_sb[:], didx_s_in[:])
                dd_sb = didxp.tile([128, TOT_DEC // 16], I16)
                nc.sync.dma_start(dd_sb[:], didx_d_in[:])
                Ms = dMp.tile([128, TOT_DEC // 128, D], BF16, tag="Ms")
                Md = dMp.tile([128, TOT_DEC // 128, D], BF16, tag="Md")
                res = doutp.tile([128, TOT_DEC // 128], F32)
                NG_DEC = c.NCH * c.NCH
                for g in range(NG_DEC):
                    chs, chd = g // c.NCH, g % c.NCH
                    off16 = g * B_dec // 16
                    coff = g * B_dec // 128
                    ncols = B_dec // 128
                    emit_gather(
                        Ms[:, coff:coff + ncols, :],
                        tablez[chs * c.CHROWS:(chs + 1) * c.CHROWS, :],
                        ds_sb[:, off16:off16 + B_dec // 16], B_dec)
                    emit_gather(
                        Md[:, coff:coff + ncols, :],
                        tablez[chd * c.CHROWS:(chd + 1) * c.CHROWS, :],
                        dd_sb[:, off16:off16 + B_dec // 16], B_dec)
                for col in range(TOT_DEC // 128):
                    mm = dwp.tile([128, D], F32, tag="mm")
                    nc.vector.tensor_tensor(
                        mm[:], Ms[:, col, :], Md[:, col, :],
                        op=mybir.AluOpType.mult)
                    nc.vector.reduce_sum(res[:, col:col + 1], mm[:],
                                         axis=mybir.AxisListType.X)
                nc.sync.dma_start(dots_out[:], res[:])

    nc.compile()
    return nc


def assemble_output(cfg, meta, results):
    c = cfg
    slot2j = meta["slot2j"]
    out = np.zeros(c.EL, dtype=np.float32)
    for core in range(len(results)):
        d = np.asarray(results[core]["dots"], dtype=np.float32)
        flat = d.T.reshape(-1)             # slot i -> d[i%128, i//128]
        s2j = slot2j[core]
        valid = s2j >= 0
        out[s2j[valid]] = flat[valid]
    return out


def run_pipeline(x, edge_index, edge_label_index, W1, b1, W2, b2,
                 cfg=None, trace=False, tmpdir=None):
    cfg = cfg or DEFAULT
    in_maps, meta = host_prep(cfg, x, edge_index, edge_label_index,
                              W1, b1, W2, b2)
    nc = build_program(cfg, meta)
    res = run_bass_kernel_spmd(nc, in_maps, list(range(cfg.NC)),
                               trace=trace, tmpdir=tmpdir)
    return assemble_output(cfg, meta, res.results), res


def kernel(x, edge_index, edge_label_index, W1, b1, W2, b2):
    out, _ = run_pipeline(x, edge_index, edge_label_index, W1, b1, W2, b2)
    return out



# revision 15
# speedup vs baseline: 1.0496x; 1.0496x over previous
"""Trainium2 Bass kernel for nn_LinkPredictor (2-layer GCN + edge-dot decode).

Strategy (8 NeuronCores, SPMD), v2 architecture:
  - Nodes sharded: core c owns rows [c*12544, (c+1)*12544).
  - Scale folding: table rows are pre-scaled by dinv[node]; the remaining
    dinv[dst] factor is applied per-window after aggregation (it commutes
    with the right-multiplication by W).  GCN conv = relu(dinv*(agg @ W)+b),
    agg[m] = self_row[m] + sum_e table[src[e]].
  - Layer-1 table (dinv*x, bf16) is host-replicated to every core: no
    AllGather needed before layer 1.  Only 2 AllGathers total (table1, tablez).
  - Edges grouped by (dst window of 128, src chunk of 25088); slots padded
    to 128-multiples per group.  Slot -> dst position is applied via one-hot
    S tiles built on DVE with a single is_equal op (pads get dstloc=-1 so
    their S row is zero; pad gathers read row 0, finite garbage * 0 = 0).
  - Aggregation: PSUM accumulation of S_t^T @ Mt_t (PE), self term via
    identity-weights matmul of the core's own contiguous table rows.
  - Per window: transpose agg (PE), matmul aggT @ W (PE), fused
    (ps*dinv[m])+b (DVE scalar_tensor_tensor), relu (Scalar engine).
  - Decode: label edge j assigned to the core owning src; s-side gathered
    from the core's own shardz (overlaps the last AllGather), d-side from
    the all-gathered tablez; dot = tensor_tensor mult + reduce.
"""
import contextlib
import math
import numpy as np
import ml_dtypes

import concourse.bass as bass
import concourse.tile as tile
from concourse import bacc, mybir
from concourse.bass_utils import run_bass_kernel_spmd
from concourse.tile_rust import add_dep_helper

F32 = mybir.dt.float32
BF16 = mybir.dt.bfloat16
I16 = mybir.dt.int16
BF = ml_dtypes.bfloat16


class Cfg:
    def __init__(self, N=100000, E=1600000, EL=100000, D=128, ncores=8,
                 nw=98, nchunks=4, wb=7):
        self.N, self.E, self.EL, self.D, self.NC = N, E, EL, D, ncores
        self.NW = nw                      # windows (128 nodes each) per core
        self.SHARD = nw * 128             # nodes per core
        self.NP = self.SHARD * ncores     # padded node count (100352)
        assert self.NP >= N
        self.NCH = nchunks                # src chunks (int16 index range)
        self.CHROWS = self.NP // nchunks  # 25088
        assert self.CHROWS <= 32767
        self.WB = wb                      # windows per gather batch
        assert nw % wb == 0
        self.NBATCH = nw // wb


DEFAULT = Cfg()


def _wrap_idxs(idx):
    """[n] ints -> [128, n//16] int16 wrapped in 16 partitions, replicated 8x."""
    n = len(idx)
    assert n % 16 == 0
    w = np.asarray(idx, dtype=np.int16).reshape(n // 16, 16).T
    return np.ascontiguousarray(np.tile(w, (8, 1)))


def host_prep(cfg, x, edge_index, edge_label_index, W1, b1, W2, b2):
    c = cfg
    src = np.asarray(edge_index[0], dtype=np.int64)
    dst = np.asarray(edge_index[1], dtype=np.int64)
    deg = np.bincount(dst, minlength=c.N).astype(np.float64) + 1.0
    dinv = 1.0 / np.sqrt(deg)                       # [N]
    dinv_p = np.zeros(c.NP, dtype=np.float32)
    dinv_p[:c.N] = dinv.astype(np.float32)

    # ---- edge slots: group by (core, window, chunk) -----------------------
    core_of = dst // c.SHARD
    w_of = (dst % c.SHARD) // 128
    m_of = dst % 128
    ch_of = src // c.CHROWS
    key = (core_of * c.NW + w_of) * c.NCH + ch_of
    order = np.argsort(key, kind="stable")
    ngroups = c.NC * c.NW * c.NCH
    counts = np.bincount(key, minlength=ngroups).reshape(c.NC, c.NW, c.NCH)
    Bwc = 128 * np.maximum(
        1, np.ceil(counts.max(axis=0) / 128).astype(np.int64))  # [NW, NCH]
    starts = np.zeros(ngroups + 1, dtype=np.int64)
    np.cumsum(counts.reshape(-1), out=starts[1:])

    # slot order: batch -> chunk -> window (within batch) -> i
    blk_of_group = {}          # (w, ch) -> first block index in dstloc
    goff = {}                  # (b, ch) -> (slot offset, n_idx)
    pos = 0
    for b in range(c.NBATCH):
        for ch in range(c.NCH):
            run0 = pos
            for w in range(b * c.WB, (b + 1) * c.WB):
                blk_of_group[(w, ch)] = pos // 128
                pos += Bwc[w, ch]
            goff[(b, ch)] = (run0, pos - run0)
    TOT = pos
    NBLK = TOT // 128

    gidx = np.zeros((c.NC, TOT), dtype=np.int64)     # pad -> row 0 of chunk
    dstloc = np.full((c.NC, NBLK, 128), -1.0, dtype=np.float32)
    for core in range(c.NC):
        for w in range(c.NW):
            for ch in range(c.NCH):
                g = (core * c.NW + w) * c.NCH + ch
                eids = order[starts[g]:starts[g + 1]]
                n = len(eids)
                p0 = blk_of_group[(w, ch)] * 128
                gidx[core, p0:p0 + n] = src[eids] % c.CHROWS
                dl = dstloc[core].reshape(-1)
                dl[p0:p0 + n] = m_of[eids]
    # dstloc layout for SBUF [128, NBLK]: column = block, partition = slot%128
    dstloc_sb = np.ascontiguousarray(dstloc.transpose(0, 2, 1))  # [NC,128,NBLK]

    # ---- decode: label edge j -> core owning src --------------------------
    ls = np.asarray(edge_label_index[0], dtype=np.int64)
    ld = np.asarray(edge_label_index[1], dtype=np.int64)
    dcore = ls // c.SHARD
    dch = ld // c.CHROWS
    dkey = dcore * c.NCH + dch
    dorder = np.argsort(dkey, kind="stable")
    dcounts = np.bincount(dkey, minlength=c.NC * c.NCH).reshape(c.NC, c.NCH)
    Bdec = 128 * np.maximum(1, np.ceil(dcounts.max(axis=0) / 128).astype(np.int64))
    dstarts = np.zeros(c.NC * c.NCH + 1, dtype=np.int64)
    np.cumsum(dcounts.reshape(-1), out=dstarts[1:])
    doff = np.zeros(c.NCH + 1, dtype=np.int64)
    np.cumsum(Bdec, out=doff[1:])
    TOT_DEC = int(doff[-1])
    sidx = np.zeros((c.NC, TOT_DEC), dtype=np.int64)
    didx = np.zeros((c.NC, TOT_DEC), dtype=np.int64)
    slot2j = np.full((c.NC, TOT_DEC), -1, dtype=np.int64)
    for core in range(c.NC):
        for ch in range(c.NCH):
            g = core * c.NCH + ch
            js = dorder[dstarts[g]:dstarts[g + 1]]
            n = len(js)
            p0 = doff[ch]
            sidx[core, p0:p0 + n] = ls[js] % c.SHARD
            didx[core, p0:p0 + n] = ld[js] % c.CHROWS
            slot2j[core, p0:p0 + n] = js

    # ---- tensors ----------------------------------------------------------
    xp = np.zeros((c.NP, c.D), dtype=np.float32)
    xp[:c.N] = np.asarray(x, dtype=np.float32)
    table0 = (xp * dinv_p[:, None]).astype(BF)       # replicated to all cores
    dinv_w = np.ascontiguousarray(
        dinv_p.reshape(c.NC, c.NW, 128).transpose(0, 2, 1))  # [NC,128,NW]
    b1r = np.tile(np.asarray(b1, np.float32)[None, :], (128, 1))
    b2r = np.tile(np.asarray(b2, np.float32)[None, :], (128, 1))

    # own-shard table0 rows in SBUF layout [128, NW, D]: row w*128+m -> [m, w, :]
    self0 = np.ascontiguousarray(
        np.asarray(table0).reshape(c.NC, c.NW, 128, c.D).transpose(0, 2, 1, 3)
        .reshape(c.NC, 128, c.NW * c.D))

    in_maps = []
    for core in range(c.NC):
        in_maps.append({
            "table0": table0,
            "self0": self0[core],
            "W1": np.asarray(W1, dtype=np.float32).astype(BF),
            "W2": np.asarray(W2, dtype=np.float32).astype(BF),
            "b1r": b1r, "b2r": b2r,
            "dinvw": dinv_w[core],
            "gidx": _wrap_idxs(gidx[core]),
            "dstloc": dstloc_sb[core],
            "sidx": _wrap_idxs(sidx[core]),
            "didx": _wrap_idxs(didx[core]),
        })
    meta = dict(Bwc=Bwc, TOT=TOT, NBLK=NBLK, goff=goff,
                blk_of_group=blk_of_group, Bdec=Bdec, doff=doff,
                TOT_DEC=TOT_DEC, slot2j=slot2j)
    return in_maps, meta


def build_program(cfg, meta, num_cores=None):
    c = cfg
    NCores = num_cores or c.NC
    Bwc, TOT, NBLK = meta["Bwc"], meta["TOT"], meta["NBLK"]
    goff, blk_of_group = meta["goff"], meta["blk_of_group"]
    Bdec, doff, TOT_DEC = meta["Bdec"], meta["doff"], meta["TOT_DEC"]
    D = c.D
    NB_DEC = TOT_DEC // 128

    nc = bacc.Bacc("TRN2", target_bir_lowering=False, debug=False,
                   num_devices=NCores, num_swdge_queues=4)

    table0 = nc.dram_tensor("table0", [c.NP, D], BF16, kind="ExternalInput")
    self0_in = nc.dram_tensor("self0", [128, c.NW * D], BF16, kind="ExternalInput")
    W1_in = nc.dram_tensor("W1", [D, D], BF16, kind="ExternalInput")
    W2_in = nc.dram_tensor("W2", [D, D], BF16, kind="ExternalInput")
    b1_in = nc.dram_tensor("b1r", [128, D], F32, kind="ExternalInput")
    b2_in = nc.dram_tensor("b2r", [128, D], F32, kind="ExternalInput")
    dinvw_in = nc.dram_tensor("dinvw", [128, c.NW], F32, kind="ExternalInput")
    gidx_in = nc.dram_tensor("gidx", [128, TOT // 16], I16, kind="ExternalInput")
    dstloc_in = nc.dram_tensor("dstloc", [128, NBLK], F32, kind="ExternalInput")
    sidx_in = nc.dram_tensor("sidx", [128, TOT_DEC // 16], I16, kind="ExternalInput")
    didx_in = nc.dram_tensor("didx", [128, TOT_DEC // 16], I16, kind="ExternalInput")
    dots_out = nc.dram_tensor("dots", [128, NB_DEC], F32, kind="ExternalOutput")

    shard1 = nc.dram_tensor("shard1", [c.SHARD, D], BF16)
    shardz = nc.dram_tensor("shardz", [c.SHARD, D], BF16)
    table1 = nc.dram_tensor("table1", [c.NP, D], BF16)
    tablez = nc.dram_tensor("tablez", [c.NP, D], BF16)

    iota_dram = nc.inline_tensor(
        np.tile(np.arange(128, dtype=np.float32), (128, 1)).astype(BF), "iota_c")
    ident_dram = nc.inline_tensor(np.eye(128, dtype=np.float32).astype(BF), "ident_c")

    cc_sem = nc.alloc_semaphore("cc_sem")
    core_ids = list(range(NCores))

    gst = {"count": 0, "qprev": {}}

    def emit_gather(out_ap, in_ap, idx_ap, n_idx):
        q = gst["count"] % 4
        inst = nc.gpsimd.dma_gather(out_ap, in_ap, idx_ap, n_idx, n_idx, D,
                                    queue_num=q, single_packet=False)
        if q in gst["qprev"]:
            add_dep_helper(inst.ins, gst["qprev"][q].ins, sync=False,
                           reason="pin swdge queue order")
        gst["qprev"][q] = inst
        gst["count"] += 1
        return inst

    with tile.TileContext(nc) as tc:
        with contextlib.ExitStack() as es:
            const = es.enter_context(tc.tile_pool(name="const", bufs=1))
            meta_p = es.enter_context(tc.tile_pool(name="meta", bufs=1))

            w1_sb = const.tile([D, D], BF16); nc.sync.dma_start(w1_sb[:], W1_in[:])
            w2_sb = const.tile([D, D], BF16); nc.sync.dma_start(w2_sb[:], W2_in[:])
            b1_sb = const.tile([128, D], F32); nc.sync.dma_start(b1_sb[:], b1_in[:])
            b2_sb = const.tile([128, D], F32); nc.sync.dma_start(b2_sb[:], b2_in[:])
            dinv_sb = const.tile([128, c.NW], F32)
            nc.sync.dma_start(dinv_sb[:], dinvw_in[:])
            iota_sb = const.tile([128, 128], BF16)
            nc.sync.dma_start(iota_sb[:], iota_dram[:])
            ident_sb = const.tile([128, 128], BF16)
            nc.sync.dma_start(ident_sb[:], ident_dram[:])
            gidx_sb = meta_p.tile([128, TOT // 16], I16)
            nc.sync.dma_start(gidx_sb[:], gidx_in[:])
            dstloc_sb = meta_p.tile([128, NBLK], F32)
            nc.sync.dma_start(dstloc_sb[:], dstloc_in[:])
            self0_sb = meta_p.tile([128, c.NW, D], BF16)
            nc.sync.dma_start(self0_sb[:], self0_in[:])

            def all_gather(shard, table_out, n_before):
                tc.strict_bb_all_engine_barrier()
                with tc.tile_critical():
                    nc.gpsimd.collective_compute(
                        "AllGather", mybir.AluOpType.bypass,
                        replica_groups=[core_ids],
                        ins=[shard[:]], outs=[table_out[:]],
                    ).then_inc(cc_sem)
                    nc.gpsimd.wait_ge(cc_sem, n_before + 1)
                tc.strict_bb_all_engine_barrier()

            def layer(lid, table, bias_sb, shard_next, self_src, t4_keep):
                """self_src: callable w -> rhs AP for the self-contribution
                block; t4_keep: pool to retain dinv*z tiles (layer 1) or
                None (layer 2)."""
                kept = []
                with tc.tile_pool(name=f"M{lid}", bufs=2) as Mp, \
                     tc.tile_pool(name=f"S{lid}", bufs=6) as Sp, \
                     tc.tile_pool(name=f"agg{lid}", bufs=3, space="PSUM") as agp, \
                     tc.tile_pool(name=f"mm{lid}", bufs=2, space="PSUM") as mmp, \
                     tc.tile_pool(name=f"tp{lid}", bufs=2, space="PSUM") as tpp, \
                     tc.tile_pool(name=f"ev{lid}", bufs=8) as evp:
                    for b in range(c.NBATCH):
                        blk0 = goff[(b, 0)][0] // 128
                        nblk_b = sum(Bwc[w, ch] for ch in range(c.NCH)
                                     for w in range(b * c.WB, (b + 1) * c.WB)) // 128
                        Mt = Mp.tile([128, nblk_b, D], BF16, tag="M")
                        for ch in range(c.NCH):
                            off, n_idx = goff[(b, ch)]
                            emit_gather(
                                Mt[:, (off // 128) - blk0:
                                   (off + n_idx) // 128 - blk0, :],
                                table[ch * c.CHROWS:(ch + 1) * c.CHROWS, :],
                                gidx_sb[:, off // 16:(off + n_idx) // 16],
                                n_idx)
                        for w in range(b * c.WB, (b + 1) * c.WB):
                            ps = agp.tile([128, D], F32, tag="agg")
                            # self contribution, starts the accumulation
                            nc.tensor.matmul(ps[:], lhsT=ident_sb[:],
                                             rhs=self_src(w),
                                             start=True, stop=False)
                            nmm = sum(Bwc[w, ch] for ch in range(c.NCH)) // 128
                            k = 0
                            for ch in range(c.NCH):
                                gblk = blk_of_group[(w, ch)]
                                for t in range(Bwc[w, ch] // 128):
                                    S = Sp.tile([128, 128], BF16, tag="S")
                                    nc.vector.tensor_scalar(
                                        S[:], iota_sb[:],
                                        dstloc_sb[:, gblk + t:gblk + t + 1],
                                        None, mybir.AluOpType.is_equal)
                                    nc.tensor.matmul(
                                        ps[:], lhsT=S[:],
                                        rhs=Mt[:, gblk + t - blk0, :],
                                        start=False, stop=(k == nmm - 1))
                                    k += 1
                            # eviction: agg -> aggT -> @W -> *dinv+b -> relu
                            aggb = evp.tile([128, D], BF16, tag="aggb")
                            nc.scalar.activation(
                                aggb[:], ps[:], mybir.ActivationFunctionType.Copy)
                            psT = tpp.tile([128, D], BF16, tag="aggT")
                            nc.tensor.transpose(psT[:], aggb[:], ident_sb[:])
                            aggT = evp.tile([128, D], BF16, tag="aggTs")
                            nc.scalar.activation(
                                aggT[:], psT[:], mybir.ActivationFunctionType.Copy)
                            wsb = w1_sb if lid == 1 else w2_sb
                            ps2 = mmp.tile([128, D], F32, tag="mm")
                            nc.tensor.matmul(ps2[:], lhsT=aggT[:], rhs=wsb[:],
                                             start=True, stop=True)
                            sc = evp.tile([128, D], F32, tag="sc")
                            nc.vector.tensor_scalar(
                                sc[:], ps2[:], dinv_sb[:, w:w + 1], None,
                                mybir.AluOpType.mult)
                            pre = evp.tile([128, D], F32, tag="pre")
                            nc.vector.tensor_tensor(
                                pre[:], sc[:], bias_sb[:],
                                op=mybir.AluOpType.add)
                            z = evp.tile([128, D], BF16, tag="z")
                            nc.scalar.activation(
                                z[:], pre[:], mybir.ActivationFunctionType.Relu)
                            if t4_keep is not None:
                                t4 = t4_keep.tile([128, D], BF16, tag="t4")
                                nc.vector.tensor_scalar(
                                    t4[:], z[:], dinv_sb[:, w:w + 1], None,
                                    mybir.AluOpType.mult)
                                out_t = t4
                                kept.append(t4)
                            else:
                                out_t = z
                            nc.sync.dma_start(
                                shard_next[w * 128:(w + 1) * 128, :], out_t[:])
                return kept

            with tc.tile_pool(name="t4p", bufs=c.NW) as t4p:
                kept = layer(1, table0, b1_sb, shard1,
                             self_src=lambda w: self0_sb[:, w, :],
                             t4_keep=t4p)
                all_gather(shard1, table1, 0)
                layer(2, table1, b2_sb, shardz,
                      self_src=lambda w, kept=kept: kept[w][:],
                      t4_keep=None)

            # ---- decode ----------------------------------------------------
            with tc.tile_pool(name="didx", bufs=1) as didxp, \
                 tc.tile_pool(name="dM", bufs=1) as dMp, \
                 tc.tile_pool(name="dout", bufs=2) as doutp:
                ds_sb = didxp.tile([128, TOT_DEC // 16], I16)
                nc.sync.dma_start(ds_sb[:], sidx_in[:])
                dd_sb = didxp.tile([128, TOT_DEC // 16], I16)
                nc.sync.dma_start(dd_sb[:], didx_in[:])
                Ms = dMp.tile([128, NB_DEC, D], BF16, tag="Ms")
                Md = dMp.tile([128, NB_DEC, D], BF16, tag="Md")

                all_gather(shardz, tablez, 1)
                emit_gather(Ms[:, :, :], shardz[:, :], ds_sb[:, :], TOT_DEC)

                for ch in range(c.NCH):
                    off = int(doff[ch]); n_idx = int(Bdec[ch])
                    emit_gather(
                        Md[:, off // 128:(off + n_idx) // 128, :],
                        tablez[ch * c.CHROWS:(ch + 1) * c.CHROWS, :],
                        dd_sb[:, off // 16:(off + n_idx) // 16], n_idx)
                prod = doutp.tile([128, NB_DEC, D], BF16, tag="prod")
                nc.vector.tensor_tensor(prod[:], Ms[:], Md[:],
                                        op=mybir.AluOpType.mult)
                res = doutp.tile([128, NB_DEC], F32, tag="res")
                nc.vector.tensor_reduce(res[:], prod[:],
                                        axis=mybir.AxisListType.X,
                                        op=mybir.AluOpType.add)
                nc.sync.dma_start(dots_out[:], res[:])

    nc.compile()
    return nc


def assemble_output(cfg, meta, results):
    c = cfg
    slot2j = meta["slot2j"]
    out = np.zeros(c.EL, dtype=np.float32)
    for core in range(len(results)):
        d = np.asarray(results[core]["dots"], dtype=np.float32)
        flat = d.T.reshape(-1)             # slot i -> d[i%128, i//128]
        s2j = slot2j[core]
        valid = s2j >= 0
        out[s2j[valid]] = flat[valid]
    return out


def run_pipeline(x, edge_index, edge_label_index, W1, b1, W2, b2,
                 cfg=None, trace=False, tmpdir=None):
    cfg = cfg or DEFAULT
    in_maps, meta = host_prep(cfg, x, edge_index, edge_label_index,
                              W1, b1, W2, b2)
    nc = build_program(cfg, meta)
    res = run_bass_kernel_spmd(nc, in_maps, list(range(cfg.NC)),
                               trace=trace, tmpdir=tmpdir)
    return assemble_output(cfg, meta, res.results), res


def kernel(x, edge_index, edge_label_index, W1, b1, W2, b2):
    out, _ = run_pipeline(x, edge_index, edge_label_index, W1, b1, W2, b2)
    return out


# revision 16
# speedup vs baseline: 1.1656x; 1.1105x over previous
"""Trainium2 Bass kernel for nn_LinkPredictor (2-layer GCN + edge-dot decode).

Strategy (8 NeuronCores, SPMD), v2 architecture:
  - Nodes sharded: core c owns rows [c*12544, (c+1)*12544).
  - Scale folding: table rows are pre-scaled by dinv[node]; the remaining
    dinv[dst] factor is applied per-window after aggregation (it commutes
    with the right-multiplication by W).  GCN conv = relu(dinv*(agg @ W)+b),
    agg[m] = self_row[m] + sum_e table[src[e]].
  - Layer-1 table (dinv*x, bf16) is host-replicated to every core: no
    AllGather needed before layer 1.  Only 2 AllGathers total (table1, tablez).
  - Edges grouped by (dst window of 128, src chunk of 25088); slots padded
    to 128-multiples per group.  Slot -> dst position is applied via one-hot
    S tiles built on DVE with a single is_equal op (pads get dstloc=-1 so
    their S row is zero; pad gathers read row 0, finite garbage * 0 = 0).
  - Aggregation: PSUM accumulation of S_t^T @ Mt_t (PE), self term via
    identity-weights matmul of the core's own contiguous table rows.
  - Per window: transpose agg (PE), matmul aggT @ W (PE), fused
    (ps*dinv[m])+b (DVE scalar_tensor_tensor), relu (Scalar engine).
  - Decode: label edge j assigned to the core owning src; s-side gathered
    from the core's own shardz (overlaps the last AllGather), d-side from
    the all-gathered tablez; dot = tensor_tensor mult + reduce.
"""
import contextlib
import math
import numpy as np
import ml_dtypes

import concourse.bass as bass
import concourse.tile as tile
from concourse import bacc, mybir
from concourse.bass_utils import run_bass_kernel_spmd
from concourse.tile_rust import add_dep_helper

F32 = mybir.dt.float32
BF16 = mybir.dt.bfloat16
I16 = mybir.dt.int16
BF = ml_dtypes.bfloat16


class Cfg:
    def __init__(self, N=100000, E=1600000, EL=100000, D=128, ncores=8,
                 nw=98, nchunks=4, wb=7):
        self.N, self.E, self.EL, self.D, self.NC = N, E, EL, D, ncores
        self.NW = nw                      # windows (128 nodes each) per core
        self.SHARD = nw * 128             # nodes per core
        self.NP = self.SHARD * ncores     # padded node count (100352)
        assert self.NP >= N
        self.NCH = nchunks                # src chunks (int16 index range)
        self.CHROWS = self.NP // nchunks  # 25088
        assert self.CHROWS <= 32767
        self.WB = wb                      # windows per gather batch
        assert nw % wb == 0
        self.NBATCH = nw // wb


DEFAULT = Cfg()


def _wrap_idxs(idx):
    """[n] ints -> [128, n//16] int16 wrapped in 16 partitions, replicated 8x."""
    n = len(idx)
    assert n % 16 == 0
    w = np.asarray(idx, dtype=np.int16).reshape(n // 16, 16).T
    return np.ascontiguousarray(np.tile(w, (8, 1)))


def host_prep(cfg, x, edge_index, edge_label_index, W1, b1, W2, b2):
    c = cfg
    src = np.asarray(edge_index[0], dtype=np.int64)
    dst = np.asarray(edge_index[1], dtype=np.int64)
    deg = np.bincount(dst, minlength=c.N).astype(np.float64) + 1.0
    dinv = 1.0 / np.sqrt(deg)                       # [N]
    dinv_p = np.zeros(c.NP, dtype=np.float32)
    dinv_p[:c.N] = dinv.astype(np.float32)

    # ---- edge slots: group by (core, window, chunk) -----------------------
    core_of = dst // c.SHARD
    w_of = (dst % c.SHARD) // 128
    m_of = dst % 128
    ch_of = src // c.CHROWS
    key = (core_of * c.NW + w_of) * c.NCH + ch_of
    order = np.argsort(key, kind="stable")
    ngroups = c.NC * c.NW * c.NCH
    counts = np.bincount(key, minlength=ngroups).reshape(c.NC, c.NW, c.NCH)
    Bwc = 128 * np.maximum(
        1, np.ceil(counts.max(axis=0) / 128).astype(np.int64))  # [NW, NCH]
    starts = np.zeros(ngroups + 1, dtype=np.int64)
    np.cumsum(counts.reshape(-1), out=starts[1:])

    # slot order: batch -> chunk -> window (within batch) -> i
    blk_of_group = {}          # (w, ch) -> first block index in dstloc
    goff = {}                  # (b, ch) -> (slot offset, n_idx)
    pos = 0
    for b in range(c.NBATCH):
        for ch in range(c.NCH):
            run0 = pos
            for w in range(b * c.WB, (b + 1) * c.WB):
                blk_of_group[(w, ch)] = pos // 128
                pos += Bwc[w, ch]
            goff[(b, ch)] = (run0, pos - run0)
    TOT = pos
    NBLK = TOT // 128

    gidx = np.zeros((c.NC, TOT), dtype=np.int64)     # pad -> row 0 of chunk
    dstloc = np.full((c.NC, NBLK, 128), -1.0, dtype=np.float32)
    for core in range(c.NC):
        for w in range(c.NW):
            for ch in range(c.NCH):
                g = (core * c.NW + w) * c.NCH + ch
                eids = order[starts[g]:starts[g + 1]]
                n = len(eids)
                p0 = blk_of_group[(w, ch)] * 128
                gidx[core, p0:p0 + n] = src[eids] % c.CHROWS
                dl = dstloc[core].reshape(-1)
                dl[p0:p0 + n] = m_of[eids]
    # dstloc layout for SBUF [128, NBLK]: column = block, partition = slot%128
    dstloc_sb = np.ascontiguousarray(dstloc.transpose(0, 2, 1))  # [NC,128,NBLK]

    # ---- decode: label edge j -> core owning src --------------------------
    ls = np.asarray(edge_label_index[0], dtype=np.int64)
    ld = np.asarray(edge_label_index[1], dtype=np.int64)
    dcore = ls // c.SHARD
    dch = ld // c.CHROWS
    dkey = dcore * c.NCH + dch
    dorder = np.argsort(dkey, kind="stable")
    dcounts = np.bincount(dkey, minlength=c.NC * c.NCH).reshape(c.NC, c.NCH)
    Bdec = 128 * np.maximum(1, np.ceil(dcounts.max(axis=0) / 128).astype(np.int64))
    dstarts = np.zeros(c.NC * c.NCH + 1, dtype=np.int64)
    np.cumsum(dcounts.reshape(-1), out=dstarts[1:])
    doff = np.zeros(c.NCH + 1, dtype=np.int64)
    np.cumsum(Bdec, out=doff[1:])
    TOT_DEC = int(doff[-1])
    sidx = np.zeros((c.NC, TOT_DEC), dtype=np.int64)
    didx = np.zeros((c.NC, TOT_DEC), dtype=np.int64)
    slot2j = np.full((c.NC, TOT_DEC), -1, dtype=np.int64)
    for core in range(c.NC):
        for ch in range(c.NCH):
            g = core * c.NCH + ch
            js = dorder[dstarts[g]:dstarts[g + 1]]
            n = len(js)
            p0 = doff[ch]
            sidx[core, p0:p0 + n] = ls[js] % c.SHARD
            didx[core, p0:p0 + n] = ld[js] % c.CHROWS
            slot2j[core, p0:p0 + n] = js

    # ---- tensors ----------------------------------------------------------
    xp = np.zeros((c.NP, c.D), dtype=np.float32)
    xp[:c.N] = np.asarray(x, dtype=np.float32)
    table0 = (xp * dinv_p[:, None]).astype(BF)       # replicated to all cores
    dinv_w = np.ascontiguousarray(
        dinv_p.reshape(c.NC, c.NW, 128).transpose(0, 2, 1))  # [NC,128,NW]
    b1r = np.tile(np.asarray(b1, np.float32)[None, :], (128, 1))
    b2r = np.tile(np.asarray(b2, np.float32)[None, :], (128, 1))

    # own-shard table0 rows in SBUF layout [128, NW, D]: row w*128+m -> [m, w, :]
    self0 = np.ascontiguousarray(
        np.asarray(table0).reshape(c.NC, c.NW, 128, c.D).transpose(0, 2, 1, 3)
        .reshape(c.NC, 128, c.NW * c.D))

    in_maps = []
    for core in range(c.NC):
        in_maps.append({
            "table0": table0,
            "self0": self0[core],
            "W1": np.asarray(W1, dtype=np.float32).astype(BF),
            "W2": np.asarray(W2, dtype=np.float32).astype(BF),
            "b1r": b1r, "b2r": b2r,
            "dinvw": dinv_w[core],
            "gidx": _wrap_idxs(gidx[core]),
            "dstloc": dstloc_sb[core],
            "sidx": _wrap_idxs(sidx[core]),
            "didx": _wrap_idxs(didx[core]),
        })
    meta = dict(Bwc=Bwc, TOT=TOT, NBLK=NBLK, goff=goff,
                blk_of_group=blk_of_group, Bdec=Bdec, doff=doff,
                TOT_DEC=TOT_DEC, slot2j=slot2j)
    return in_maps, meta


def build_program(cfg, meta, num_cores=None):
    c = cfg
    NCores = num_cores or c.NC
    Bwc, TOT, NBLK = meta["Bwc"], meta["TOT"], meta["NBLK"]
    goff, blk_of_group = meta["goff"], meta["blk_of_group"]
    Bdec, doff, TOT_DEC = meta["Bdec"], meta["doff"], meta["TOT_DEC"]
    D = c.D
    NB_DEC = TOT_DEC // 128

    nc = bacc.Bacc("TRN2", target_bir_lowering=False, debug=False,
                   num_devices=NCores, num_swdge_queues=4)

    table0 = nc.dram_tensor("table0", [c.NP, D], BF16, kind="ExternalInput")
    self0_in = nc.dram_tensor("self0", [128, c.NW * D], BF16, kind="ExternalInput")
    W1_in = nc.dram_tensor("W1", [D, D], BF16, kind="ExternalInput")
    W2_in = nc.dram_tensor("W2", [D, D], BF16, kind="ExternalInput")
    b1_in = nc.dram_tensor("b1r", [128, D], F32, kind="ExternalInput")
    b2_in = nc.dram_tensor("b2r", [128, D], F32, kind="ExternalInput")
    dinvw_in = nc.dram_tensor("dinvw", [128, c.NW], F32, kind="ExternalInput")
    gidx_in = nc.dram_tensor("gidx", [128, TOT // 16], I16, kind="ExternalInput")
    dstloc_in = nc.dram_tensor("dstloc", [128, NBLK], F32, kind="ExternalInput")
    sidx_in = nc.dram_tensor("sidx", [128, TOT_DEC // 16], I16, kind="ExternalInput")
    didx_in = nc.dram_tensor("didx", [128, TOT_DEC // 16], I16, kind="ExternalInput")
    dots_out = nc.dram_tensor("dots", [128, NB_DEC], F32, kind="ExternalOutput")

    shard1 = nc.dram_tensor("shard1", [c.SHARD, D], BF16)
    shardz = nc.dram_tensor("shardz", [c.SHARD, D], BF16)
    table1 = nc.dram_tensor("table1", [c.NP, D], BF16)
    tablez = nc.dram_tensor("tablez", [c.NP, D], BF16)

    iota_dram = nc.inline_tensor(
        np.tile(np.arange(128, dtype=np.float32), (128, 1)).astype(BF), "iota_c")
    ident_dram = nc.inline_tensor(np.eye(128, dtype=np.float32).astype(BF), "ident_c")

    cc_sem = nc.alloc_semaphore("cc_sem")
    core_ids = list(range(NCores))

    gst = {"count": 0, "qprev": {}}

    def emit_gather(out_ap, in_ap, idx_ap, n_idx):
        q = gst["count"] % 4
        inst = nc.gpsimd.dma_gather(out_ap, in_ap, idx_ap, n_idx, n_idx, D,
                                    queue_num=q, single_packet=False)
        if q in gst["qprev"]:
            add_dep_helper(inst.ins, gst["qprev"][q].ins, sync=False,
                           reason="pin swdge queue order")
        gst["qprev"][q] = inst
        gst["count"] += 1
        return inst

    with tile.TileContext(nc) as tc:
        with contextlib.ExitStack() as es:
            const = es.enter_context(tc.tile_pool(name="const", bufs=1))
            meta_p = es.enter_context(tc.tile_pool(name="meta", bufs=1))

            w1_sb = const.tile([D, D], BF16); nc.sync.dma_start(w1_sb[:], W1_in[:])
            w2_sb = const.tile([D, D], BF16); nc.sync.dma_start(w2_sb[:], W2_in[:])
            b1_sb = const.tile([128, D], F32); nc.sync.dma_start(b1_sb[:], b1_in[:])
            b2_sb = const.tile([128, D], F32); nc.sync.dma_start(b2_sb[:], b2_in[:])
            dinv_sb = const.tile([128, c.NW], F32)
            nc.sync.dma_start(dinv_sb[:], dinvw_in[:])
            iota_sb = const.tile([128, 128], BF16)
            nc.sync.dma_start(iota_sb[:], iota_dram[:])
            ident_sb = const.tile([128, 128], BF16)
            nc.sync.dma_start(ident_sb[:], ident_dram[:])
            gidx_sb = meta_p.tile([128, TOT // 16], I16)
            nc.sync.dma_start(gidx_sb[:], gidx_in[:])
            dstloc_sb = meta_p.tile([128, NBLK], F32)
            nc.sync.dma_start(dstloc_sb[:], dstloc_in[:])
            self0_sb = meta_p.tile([128, c.NW, D], BF16)
            nc.sync.dma_start(self0_sb[:], self0_in[:])

            def all_gather(shard, table_out, n_before):
                tc.strict_bb_all_engine_barrier()
                with tc.tile_critical():
                    nc.gpsimd.collective_compute(
                        "AllGather", mybir.AluOpType.bypass,
                        replica_groups=[core_ids],
                        ins=[shard[:]], outs=[table_out[:]],
                    ).then_inc(cc_sem)
                    nc.gpsimd.wait_ge(cc_sem, n_before + 1)
                tc.strict_bb_all_engine_barrier()

            def layer(lid, table, bias_sb, shard_next, self_src, t4_keep):
                """self_src: callable w -> rhs AP for the self-contribution
                block; t4_keep: pool to retain dinv*z tiles (layer 1) or
                None (layer 2)."""
                kept = []
                with tc.tile_pool(name=f"M{lid}", bufs=2) as Mp, \
                     tc.tile_pool(name=f"S{lid}", bufs=6) as Sp, \
                     tc.tile_pool(name=f"agg{lid}", bufs=3, space="PSUM") as agp, \
                     tc.tile_pool(name=f"mm{lid}", bufs=2, space="PSUM") as mmp, \
                     tc.tile_pool(name=f"tp{lid}", bufs=2, space="PSUM") as tpp, \
                     tc.tile_pool(name=f"ev{lid}", bufs=8) as evp:
                    for b in range(c.NBATCH):
                        blk0 = goff[(b, 0)][0] // 128
                        nblk_b = sum(Bwc[w, ch] for ch in range(c.NCH)
                                     for w in range(b * c.WB, (b + 1) * c.WB)) // 128
                        Mt = Mp.tile([128, nblk_b, D], BF16, tag="M")
                        with tc.high_priority():
                            for ch in range(c.NCH):
                                off, n_idx = goff[(b, ch)]
                                half = (n_idx // 256) * 128
                                for o0, nn in ((0, half), (half, n_idx - half)):
                                    if nn == 0:
                                        continue
                                    o = off + o0
                                    emit_gather(
                                        Mt[:, o // 128 - blk0:
                                           (o + nn) // 128 - blk0, :],
                                        table[ch * c.CHROWS:
                                              (ch + 1) * c.CHROWS, :],
                                        gidx_sb[:, o // 16:(o + nn) // 16],
                                        nn)
                        for w in range(b * c.WB, (b + 1) * c.WB):
                            ps = agp.tile([128, D], F32, tag="agg")
                            # self contribution, starts the accumulation
                            nc.tensor.matmul(ps[:], lhsT=ident_sb[:],
                                             rhs=self_src(w),
                                             start=True, stop=False)
                            nmm = sum(Bwc[w, ch] for ch in range(c.NCH)) // 128
                            k = 0
                            for ch in range(c.NCH):
                                gblk = blk_of_group[(w, ch)]
                                for t in range(Bwc[w, ch] // 128):
                                    S = Sp.tile([128, 128], BF16, tag="S")
                                    nc.vector.tensor_scalar(
                                        S[:], iota_sb[:],
                                        dstloc_sb[:, gblk + t:gblk + t + 1],
                                        None, mybir.AluOpType.is_equal)
                                    nc.tensor.matmul(
                                        ps[:], lhsT=S[:],
                                        rhs=Mt[:, gblk + t - blk0, :],
                                        start=False, stop=(k == nmm - 1))
                                    k += 1
                            # eviction: agg -> aggT -> @W -> *dinv+b -> relu
                            aggb = evp.tile([128, D], BF16, tag="aggb")
                            nc.scalar.activation(
                                aggb[:], ps[:], mybir.ActivationFunctionType.Copy)
                            psT = tpp.tile([128, D], BF16, tag="aggT")
                            nc.tensor.transpose(psT[:], aggb[:], ident_sb[:])
                            aggT = evp.tile([128, D], BF16, tag="aggTs")
                            nc.scalar.activation(
                                aggT[:], psT[:], mybir.ActivationFunctionType.Copy)
                            wsb = w1_sb if lid == 1 else w2_sb
                            ps2 = mmp.tile([128, D], F32, tag="mm")
                            nc.tensor.matmul(ps2[:], lhsT=aggT[:], rhs=wsb[:],
                                             start=True, stop=True)
                            sc = evp.tile([128, D], F32, tag="sc")
                            nc.vector.tensor_scalar(
                                sc[:], ps2[:], dinv_sb[:, w:w + 1], None,
                                mybir.AluOpType.mult)
                            pre = evp.tile([128, D], F32, tag="pre")
                            nc.vector.tensor_tensor(
                                pre[:], sc[:], bias_sb[:],
                                op=mybir.AluOpType.add)
                            z = evp.tile([128, D], BF16, tag="z")
                            nc.scalar.activation(
                                z[:], pre[:], mybir.ActivationFunctionType.Relu)
                            if t4_keep is not None:
                                t4 = t4_keep.tile([128, D], BF16, tag="t4")
                                nc.vector.tensor_scalar(
                                    t4[:], z[:], dinv_sb[:, w:w + 1], None,
                                    mybir.AluOpType.mult)
                                out_t = t4
                                kept.append(t4)
                            else:
                                out_t = z
                            nc.sync.dma_start(
                                shard_next[w * 128:(w + 1) * 128, :], out_t[:])
                return kept

            with tc.tile_pool(name="t4p", bufs=c.NW) as t4p:
                kept = layer(1, table0, b1_sb, shard1,
                             self_src=lambda w: self0_sb[:, w, :],
                             t4_keep=t4p)
                all_gather(shard1, table1, 0)
                layer(2, table1, b2_sb, shardz,
                      self_src=lambda w, kept=kept: kept[w][:],
                      t4_keep=None)

            # ---- decode ----------------------------------------------------
            with tc.tile_pool(name="didx", bufs=1) as didxp, \
                 tc.tile_pool(name="dM", bufs=1) as dMp, \
                 tc.tile_pool(name="dout", bufs=2) as doutp:
                ds_sb = didxp.tile([128, TOT_DEC // 16], I16)
                nc.sync.dma_start(ds_sb[:], sidx_in[:])
                dd_sb = didxp.tile([128, TOT_DEC // 16], I16)
                nc.sync.dma_start(dd_sb[:], didx_in[:])
                Ms = dMp.tile([128, NB_DEC, D], BF16, tag="Ms")
                Md = dMp.tile([128, NB_DEC, D], BF16, tag="Md")

                all_gather(shardz, tablez, 1)
                emit_gather(Ms[:, :, :], shardz[:, :], ds_sb[:, :], TOT_DEC)

                for ch in range(c.NCH):
                    off = int(doff[ch]); n_idx = int(Bdec[ch])
                    emit_gather(
                        Md[:, off // 128:(off + n_idx) // 128, :],
                        tablez[ch * c.CHROWS:(ch + 1) * c.CHROWS, :],
                        dd_sb[:, off // 16:(off + n_idx) // 16], n_idx)
                prod = doutp.tile([128, NB_DEC, D], BF16, tag="prod")
                nc.vector.tensor_tensor(prod[:], Ms[:], Md[:],
                                        op=mybir.AluOpType.mult)
                res = doutp.tile([128, NB_DEC], F32, tag="res")
                nc.vector.tensor_reduce(res[:], prod[:],
                                        axis=mybir.AxisListType.X,
                                        op=mybir.AluOpType.add)
                nc.sync.dma_start(dots_out[:], res[:])

    nc.compile()
    return nc


def assemble_output(cfg, meta, results):
    c = cfg
    slot2j = meta["slot2j"]
    out = np.zeros(c.EL, dtype=np.float32)
    for core in range(len(results)):
        d = np.asarray(results[core]["dots"], dtype=np.float32)
        flat = d.T.reshape(-1)             # slot i -> d[i%128, i//128]
        s2j = slot2j[core]
        valid = s2j >= 0
        out[s2j[valid]] = flat[valid]
    return out


def run_pipeline(x, edge_index, edge_label_index, W1, b1, W2, b2,
                 cfg=None, trace=False, tmpdir=None):
    cfg = cfg or DEFAULT
    in_maps, meta = host_prep(cfg, x, edge_index, edge_label_index,
                              W1, b1, W2, b2)
    nc = build_program(cfg, meta)
    res = run_bass_kernel_spmd(nc, in_maps, list(range(cfg.NC)),
                               trace=trace, tmpdir=tmpdir)
    return assemble_output(cfg, meta, res.results), res


def kernel(x, edge_index, edge_label_index, W1, b1, W2, b2):
    out, _ = run_pipeline(x, edge_index, edge_label_index, W1, b1, W2, b2)
    return out
